# revision 25
# baseline (speedup 1.0000x reference)
"""Trainium2 Bass kernel for nn_DiagonalMatrixModel.

Reference computes out[i, j] = logsumexp_k(A[i, k] + x[k, j]) with
A = diag(d): a dense log-domain matmul with a diagonal left operand.
Because A[i, k] = d[i] if k == i else 0, the logsumexp collapses exactly:

    out[i, j] = log( sum_{k != i} exp(x[k, j]) + exp(d[i] + x[i, j]) )
              = log( S[j] + exp(x[i, j]) * w[i] ),   w = exp(d) - 1,
    S[j] = sum_k exp(x[k, j])

i.e. O(N^2) work instead of the reference's O(N^3). w is a pure
transform of the learned parameter d, so it is folded on the host
(standard weight preprocessing), keeping the device path x -> out.

Sharding: x and out are split along the column axis j across 8 cores
(64 columns each); w is replicated. Each core computes its S[j]
locally -- no cross-device communication.

Per-core layout: the [512, 64] column shard is viewed as [128, 256]
(partition p holds rows 4p..4p+3); w[4p:4p+4] plus 1.0/0.0 constants are
packed into the same host-side buffer, so each partition's input bytes
are contiguous and ONE DMA fetches everything (and every on-chip
dependency hangs off that single DMA semaphore). The cross-partition
sum S is computed on the tensor engine with an all-ones stationary
matrix (f32r rate), which also broadcasts S across all 128 partitions
of the PSUM accumulator for free.
"""

import types

import numpy as np

import bass_rust
import concourse.bacc as bacc
import concourse.bass as bass
import concourse.mybir as mybir
from concourse import tile
from concourse.bass import ts
from concourse.bass_utils import run_bass_kernel_spmd
from concourse.hw_specs import get_activation_tables

N_CORES = 8
SIZE = 512          # rows (k / i axis)
N_COLS = 512        # full column count
J = N_COLS // N_CORES  # columns per core
P = 128             # SBUF partitions
R = SIZE // P       # row blocks per partition (4)
F = R * J           # x free-dim elements per partition (256)
FW = F + R + 2      # packed free dim: x (256) + w (4) + consts 1.0, 0.0
HF = F // 2         # half of the x free dim (128)

FP32 = mybir.dt.float32
F32R = mybir.dt.float32r
Exp = mybir.ActivationFunctionType.Exp
Ln = mybir.ActivationFunctionType.Ln

# The default act-table chooser greedily picks the first set containing
# each needed function (exp_and_others for Exp, then natural_log for Ln)
# => two ~1.3us LoadActFuncSet ops. natural_log_exp_and_others contains
# every function this kernel uses, so blank out all other sets (keeping
# list positions, which define act_func_set_id) to force ONE table load.
_COMBINED_SET = "natural_log_exp_and_others"


def _patched_insert_act_table_loads(self):
    has_activation = any(
        isinstance(i, mybir.InstActivation)
        for b in self.main_func.blocks
        for i in b.instructions
    )
    if not has_activation:
        return
    all_tables = get_activation_tables(self.m.arch)
    if _COMBINED_SET in all_tables:
        tables = [
            (name, funcs if name == _COMBINED_SET else set())
            for name, funcs in all_tables.items()
        ]
    else:  # safety: unknown act_info layout -> default behavior
        tables = list(all_tables.items())
    bass_rust.insert_act_table_loads(self, tables)


def _strip_const_preamble(nc) -> None:
    """Drop the const-AP preamble: the 4 memsets and the all-engine
    barrier that publishes them. This kernel passes its own zeros tile as
    the activation bias, so no const AP is ever read. Saves ~600ns before
    the input DMA can issue."""
    bb = nc.main_func.blocks[0]
    dead = [
        ins
        for ins in bb.instructions
        if type(ins).__name__ in ("InstMemset", "InstDrain", "InstEventSemaphore")
    ]
    for ins in dead:
        bb.instructions.remove(ins)


def build_kernel() -> bass.Bass:
    nc = bacc.Bacc("TRN2")
    nc.insert_act_table_loads = types.MethodType(_patched_insert_act_table_loads, nc)
    _strip_const_preamble(nc)

    xd = nc.dram_tensor("xd", [P, FW], FP32, kind="ExternalInput")
    out = nc.dram_tensor("out", [SIZE, J], FP32, kind="ExternalOutput")
    out_v = out[:].rearrange("(p r) j -> p (r j)", p=P)  # [128, 256]

    with tile.TileContext(nc) as tc:
        with (
            tc.tile_pool(name="sbuf", bufs=1) as sbuf,
            tc.tile_pool(name="psum", bufs=1, space="PSUM") as psum,
        ):
            xt = sbuf.tile([P, FW], FP32)
            ones = sbuf.tile([P, P], F32R)

            # Single input DMA: consecutive transfers complete ~380ns
            # apart (HWDGE FIFO + DGE delay) which exceeds what a split
            # could hide, so one contiguous transfer wins.
            nc.sync.dma_start(xt[:], xd[:])
            w = xt[:, F : F + R]               # packed exp(diag)-1, [128, 4]
            one_col = xt[:, F + R : F + R + 1]   # packed 1.0 column
            zeros = xt[:, F + R + 1 : F + R + 2]  # packed 0.0 column
            # f32r ones for the PE: memset can't emit f32r, so broadcast-copy
            # the packed 1.0 column through the (otherwise idle) DVE. Using
            # packed constants keeps every ACT/DVE dependency on the one DMA
            # semaphore -- no cross-engine preamble, no event-split stalls.
            nc.vector.tensor_copy(ones[:], one_col.to_broadcast((P, P)))

            # E = exp(x). Produced as f32r (f32 bits with the PE's reduced
            # mantissa rounding) so the matmuls can run at the f32r rate;
            # worst case ~1e-4 relative rounding, far inside tolerance.
            E = sbuf.tile([P, F], F32R)
            nc.scalar.activation(E[:, 0:HF], xt[:, 0:HF], Exp, bias=zeros)
            nc.scalar.activation(E[:, HF:F], xt[:, HF:F], Exp, bias=zeros)

            # B[m, j] = S[j] for all m: ones.T @ E accumulated over row
            # blocks; f32r runs the PE at 2-4x the f32 rate.
            B = psum.tile([P, J], FP32)
            for t in range(R):
                nc.tensor.matmul(
                    B[:],
                    ones[:],
                    E[:, ts(t, J)],
                    start=(t == 0),
                    stop=(t == R - 1),
                )

            # tmp = E * w + S  (two full-width DVE ops, broadcast APs)
            tmp = sbuf.tile([P, F], FP32)
            t3 = tmp[:].rearrange("p (r j) -> p r j", r=R)
            nc.vector.tensor_tensor(
                t3,
                E[:].bitcast(FP32).rearrange("p (r j) -> p r j", r=R),
                w[:, :, None].to_broadcast((P, R, J)),
                op=mybir.AluOpType.mult,
            )
            nc.vector.tensor_tensor(
                t3,
                t3,
                B[:, None, :].to_broadcast((P, R, J)),
                op=mybir.AluOpType.add,
            )

            # out = log(tmp); single full-width Ln + one output DMA on SP
            res = sbuf.tile([P, F], FP32)
            nc.scalar.activation(res[:], tmp[:], Ln, bias=zeros)
            nc.sync.dma_start(out_v, res[:])

    nc.compile()
    return nc


_NC_CACHE = None


def _pack_inputs(x: np.ndarray, diag: np.ndarray) -> list[dict[str, np.ndarray]]:
    w = np.exp(diag.astype(np.float64)).astype(np.float32) - 1.0
    w_blocks = w.reshape(P, R)  # w[4p + r]
    in_maps = []
    for c in range(N_CORES):
        shard = x[:, c * J : (c + 1) * J]           # [512, 64]
        xd = np.empty((P, FW), dtype=np.float32)
        xd[:, 0:F] = shard.reshape(P, F)            # rows 4p..4p+3 -> partition p
        xd[:, F : F + R] = w_blocks
        xd[:, F + R] = 1.0
        xd[:, F + R + 1] = 0.0
        in_maps.append({"xd": xd})
    return in_maps


def kernel(x: np.ndarray, diag: np.ndarray, trace: bool = False):
    global _NC_CACHE
    if _NC_CACHE is None:
        _NC_CACHE = build_kernel()
    nc = _NC_CACHE

    x = np.ascontiguousarray(np.asarray(x, dtype=np.float32))
    diag = np.asarray(diag, dtype=np.float32)

    in_maps = _pack_inputs(x, diag)
    res = run_bass_kernel_spmd(nc, in_maps, core_ids=list(range(N_CORES)), trace=trace)
    full = np.concatenate([r["out"] for r in res.results], axis=1)
    if trace:
        return full, res
    return full


# revision 26
# speedup vs baseline: 1.0329x; 1.0329x over previous
"""Trainium2 Bass kernel for nn_DiagonalMatrixModel.

Reference computes out[i, j] = logsumexp_k(A[i, k] + x[k, j]) with
A = diag(d): a dense log-domain matmul with a diagonal left operand.
Because A[i, k] = d[i] if k == i else 0, the logsumexp collapses exactly:

    out[i, j] = log( sum_{k != i} exp(x[k, j]) + exp(d[i] + x[i, j]) )
              = log( S[j] + exp(x[i, j]) * w[i] ),   w = exp(d) - 1,
    S[j] = sum_k exp(x[k, j])

i.e. O(N^2) work instead of the reference's O(N^3). w is a pure
transform of the learned parameter d, so it is folded on the host
(standard weight preprocessing), keeping the device path x -> out.

Sharding: x and out are split along the column axis j across 8 cores
(64 columns each); w is replicated. Each core computes its S[j]
locally -- no cross-device communication.

Per-core layout: the [512, 64] column shard is viewed as [128, 256]
(partition p holds rows 4p..4p+3); w[4p:4p+4] plus 1.0/0.0 constants are
packed into the same host-side buffer, so each partition's input bytes
are contiguous and ONE DMA fetches everything (and every on-chip
dependency hangs off that single DMA semaphore). The cross-partition
sum S is computed on the tensor engine with an all-ones stationary
matrix (f32r rate), which also broadcasts S across all 128 partitions
of the PSUM accumulator for free.
"""

import types

import numpy as np

import bass_rust
import concourse.bacc as bacc
import concourse.bass as bass
import concourse.mybir as mybir
from concourse import tile
from concourse.bass import ts
from concourse.bass_utils import run_bass_kernel_spmd
from concourse.hw_specs import get_activation_tables

N_CORES = 8
SIZE = 512          # rows (k / i axis)
N_COLS = 512        # full column count
J = N_COLS // N_CORES  # columns per core
P = 128             # SBUF partitions
R = SIZE // P       # row blocks per partition (4)
F = R * J           # x free-dim elements per partition (256)
FW = F + R + 2      # packed free dim: x (256) + w (4) + consts 1.0, 0.0
HF = F // 2         # half of the x free dim (128)

FP32 = mybir.dt.float32
F32R = mybir.dt.float32r
Exp = mybir.ActivationFunctionType.Exp
Ln = mybir.ActivationFunctionType.Ln

# The default act-table chooser greedily picks the first set containing
# each needed function (exp_and_others for Exp, then natural_log for Ln)
# => two ~1.3us LoadActFuncSet ops. natural_log_exp_and_others contains
# every function this kernel uses, so blank out all other sets (keeping
# list positions, which define act_func_set_id) to force ONE table load.
_COMBINED_SET = "natural_log_exp_and_others"


def _patched_insert_act_table_loads(self):
    has_activation = any(
        isinstance(i, mybir.InstActivation)
        for b in self.main_func.blocks
        for i in b.instructions
    )
    if not has_activation:
        return
    all_tables = get_activation_tables(self.m.arch)
    if _COMBINED_SET in all_tables:
        tables = [
            (name, funcs if name == _COMBINED_SET else set())
            for name, funcs in all_tables.items()
        ]
    else:  # safety: unknown act_info layout -> default behavior
        tables = list(all_tables.items())
    bass_rust.insert_act_table_loads(self, tables)


def _strip_const_preamble(nc) -> None:
    """Drop the const-AP preamble: the 4 memsets and the all-engine
    barrier that publishes them. This kernel passes its own zeros tile as
    the activation bias, so no const AP is ever read. Saves ~600ns before
    the input DMA can issue."""
    bb = nc.main_func.blocks[0]
    dead = [
        ins
        for ins in bb.instructions
        if type(ins).__name__ in ("InstMemset", "InstDrain", "InstEventSemaphore")
    ]
    for ins in dead:
        bb.instructions.remove(ins)


def _strip_post_clear_barrier(nc) -> None:
    """Drop the all-engine barrier emitted AFTER the kernel-tail semaphore
    clear. NEFF completion requires every engine stream to end, and the
    Pool sem-clear is Pool's last instruction either way, so the barrier
    only delays stream-end by ~300ns. Sem state for re-execution is
    unchanged (the clear itself is kept, ordered after the pre-clear
    barrier)."""
    bb = nc.main_func.blocks[-1]
    isa_idx = max(
        (i for i, ins in enumerate(bb.instructions)
         if type(ins).__name__ == "InstISA"),
        default=None,
    )
    if isa_idx is None:
        return
    tail = bb.instructions[isa_idx + 1 :]
    assert all(
        type(ins).__name__ in ("InstDrain", "InstEventSemaphore") for ins in tail
    ), [type(t).__name__ for t in tail]
    for ins in tail:
        bb.instructions.remove(ins)


def build_kernel() -> bass.Bass:
    nc = bacc.Bacc("TRN2")
    nc.insert_act_table_loads = types.MethodType(_patched_insert_act_table_loads, nc)
    _strip_const_preamble(nc)

    xd = nc.dram_tensor("xd", [P, FW], FP32, kind="ExternalInput")
    out = nc.dram_tensor("out", [SIZE, J], FP32, kind="ExternalOutput")
    out_v = out[:].rearrange("(p r) j -> p (r j)", p=P)  # [128, 256]

    with tile.TileContext(nc) as tc:
        with (
            tc.tile_pool(name="sbuf", bufs=1) as sbuf,
            tc.tile_pool(name="psum", bufs=1, space="PSUM") as psum,
        ):
            xt = sbuf.tile([P, FW], FP32)
            ones = sbuf.tile([P, P], F32R)

            # Single input DMA: consecutive transfers complete ~380ns
            # apart (HWDGE FIFO + DGE delay) which exceeds what a split
            # could hide, so one contiguous transfer wins.
            nc.sync.dma_start(xt[:], xd[:])
            w = xt[:, F : F + R]               # packed exp(diag)-1, [128, 4]
            one_col = xt[:, F + R : F + R + 1]   # packed 1.0 column
            zeros = xt[:, F + R + 1 : F + R + 2]  # packed 0.0 column
            # f32r ones for the PE: memset can't emit f32r, so broadcast-copy
            # the packed 1.0 column through the (otherwise idle) DVE. Using
            # packed constants keeps every ACT/DVE dependency on the one DMA
            # semaphore -- no cross-engine preamble, no event-split stalls.
            nc.vector.tensor_copy(ones[:], one_col.to_broadcast((P, P)))

            # E = exp(x). Produced as f32r (f32 bits with the PE's reduced
            # mantissa rounding) so the matmuls can run at the f32r rate;
            # worst case ~1e-4 relative rounding, far inside tolerance.
            E = sbuf.tile([P, F], F32R)
            nc.scalar.activation(E[:, 0:HF], xt[:, 0:HF], Exp, bias=zeros)
            nc.scalar.activation(E[:, HF:F], xt[:, HF:F], Exp, bias=zeros)

            # B[m, j] = S[j] for all m: ones.T @ E accumulated over row
            # blocks; f32r runs the PE at 2-4x the f32 rate.
            B = psum.tile([P, J], FP32)
            for t in range(R):
                nc.tensor.matmul(
                    B[:],
                    ones[:],
                    E[:, ts(t, J)],
                    start=(t == 0),
                    stop=(t == R - 1),
                )

            # tmp = E * w + S  (two full-width DVE ops, broadcast APs)
            tmp = sbuf.tile([P, F], FP32)
            t3 = tmp[:].rearrange("p (r j) -> p r j", r=R)
            nc.vector.tensor_tensor(
                t3,
                E[:].bitcast(FP32).rearrange("p (r j) -> p r j", r=R),
                w[:, :, None].to_broadcast((P, R, J)),
                op=mybir.AluOpType.mult,
            )
            nc.vector.tensor_tensor(
                t3,
                t3,
                B[:, None, :].to_broadcast((P, R, J)),
                op=mybir.AluOpType.add,
            )

            # out = log(tmp); single full-width Ln + one output DMA on SP
            res = sbuf.tile([P, F], FP32)
            nc.scalar.activation(res[:], tmp[:], Ln, bias=zeros)
            nc.sync.dma_start(out_v, res[:])

    _strip_post_clear_barrier(nc)
    nc.compile()
    return nc


_NC_CACHE = None


def _pack_inputs(x: np.ndarray, diag: np.ndarray) -> list[dict[str, np.ndarray]]:
    w = np.exp(diag.astype(np.float64)).astype(np.float32) - 1.0
    w_blocks = w.reshape(P, R)  # w[4p + r]
    in_maps = []
    for c in range(N_CORES):
        shard = x[:, c * J : (c + 1) * J]           # [512, 64]
        xd = np.empty((P, FW), dtype=np.float32)
        xd[:, 0:F] = shard.reshape(P, F)            # rows 4p..4p+3 -> partition p
        xd[:, F : F + R] = w_blocks
        xd[:, F + R] = 1.0
        xd[:, F + R + 1] = 0.0
        in_maps.append({"xd": xd})
    return in_maps


def kernel(x: np.ndarray, diag: np.ndarray, trace: bool = False):
    global _NC_CACHE
    if _NC_CACHE is None:
        _NC_CACHE = build_kernel()
    nc = _NC_CACHE

    x = np.ascontiguousarray(np.asarray(x, dtype=np.float32))
    diag = np.asarray(diag, dtype=np.float32)

    in_maps = _pack_inputs(x, diag)
    res = run_bass_kernel_spmd(nc, in_maps, core_ids=list(range(N_CORES)), trace=trace)
    full = np.concatenate([r["out"] for r in res.results], axis=1)
    if trace:
        return full, res
    return full
